# revision 1
# baseline (speedup 1.0000x reference)
"""Trainium2 Bass kernel for nn_EnsembleBeliefs (batched scatter-add into
per-estimator belief tables).

  new_a[e, r] = a[e, r] + sum_{s: samples_regions[s,e]==r} da[s]   (same for b)

Sharding: estimator-parallel across 8 NeuronCores (16 estimators each, no
cross-core communication).

Per-core algorithm (PE one-hot matmul scatter):
  region r = hi*512 + lo  (hi in [0,128) -> PSUM partition, lo in [0,512))
  For each 128-sample chunk:
    cmp[s, h] = (hi_s == h)        one-hot, bf16 (exact 0/1)
    W[s, h]   = cmp * da_s         da split into two bf16 parts for fp32-level
    X[s, l]   = (lo_s == l)        accuracy (hi + residual)
    psum[h, l] += W^T @ X          TensorE matmul, fp32 PSUM accumulation
  After all chunks psum holds the full delta table; out = a + psum.
"""
import numpy as np
import concourse.bass as bass
import concourse.bacc as bacc
import concourse.tile as tile
from concourse import mybir
from concourse.bass_utils import run_bass_kernel_spmd

F32 = mybir.dt.float32
BF16 = mybir.dt.bfloat16
I32 = mybir.dt.int32
I16 = mybir.dt.int16

E = 128          # estimators
R = 65536        # regions per estimator
S = 100000       # update samples
N_CORES = 8
E_PC = E // N_CORES          # 16 estimators per core
S_PAD = 100096               # S padded to a multiple of 128 (da/db padded with 0)
NCH = S_PAD // 128           # 782 sample chunks
G_BLK = 8                    # chunks per batched DVE op
EXACT = True                 # split da/db into 2 bf16 parts (4 matmuls/chunk)

LAST_RESULTS = None          # BassKernelResults of the most recent run (for test harness)
_CACHED_NC = None


def _build_kernel():
    nc = bacc.Bacc("TRN2", target_bir_lowering=False, debug=False,
                   num_devices=N_CORES)
    sr_d = nc.dram_tensor("sr", [E_PC, 128, NCH], I32, kind="ExternalInput")
    da_d = nc.dram_tensor("da_l", [128, NCH], F32, kind="ExternalInput")
    db_d = nc.dram_tensor("db_l", [128, NCH], F32, kind="ExternalInput")
    a_d = nc.dram_tensor("a", [E_PC, 128, 512], F32, kind="ExternalInput")
    b_d = nc.dram_tensor("b", [E_PC, 128, 512], F32, kind="ExternalInput")
    io128_d = nc.dram_tensor("iota128", [128, 128], I16, kind="ExternalInput")
    io512_d = nc.dram_tensor("iota512", [128, 512], I16, kind="ExternalInput")
    oa_d = nc.dram_tensor("out_a", [E_PC, 128, 512], F32, kind="ExternalOutput")
    ob_d = nc.dram_tensor("out_b", [E_PC, 128, 512], F32, kind="ExternalOutput")

    OP = mybir.AluOpType

    with tile.TileContext(nc) as tc:
        with (
            tc.tile_pool(name="const", bufs=1) as constp,
            tc.tile_pool(name="dprep", bufs=1) as dprep,
            tc.tile_pool(name="est", bufs=2) as estp,
            tc.tile_pool(name="blk", bufs=3) as blkp,
            tc.tile_pool(name="outp", bufs=3) as outp,
            tc.tile_pool(name="psum", bufs=2, space=bass.MemorySpace.PSUM) as psump,
        ):
            io128 = constp.tile([128, 128], I16)
            io512 = constp.tile([128, 512], I16)
            nc.sync.dma_start(io128[:, :], io128_d.ap()[:, :])
            nc.sync.dma_start(io512[:, :], io512_d.ap()[:, :])

            da32 = dprep.tile([128, NCH], F32, tag="d32")
            db32 = dprep.tile([128, NCH], F32, tag="d32b")
            nc.sync.dma_start(da32[:, :], da_d.ap()[:, :])
            nc.sync.dma_start(db32[:, :], db_d.ap()[:, :])
            parts = []   # (bf16 value tile, table id)  0 = a, 1 = b
            da_h = dprep.tile([128, NCH], BF16, tag="dah")
            db_h = dprep.tile([128, NCH], BF16, tag="dbh")
            nc.vector.tensor_copy(da_h[:, :], da32[:, :])
            nc.vector.tensor_copy(db_h[:, :], db32[:, :])
            parts += [(da_h, 0), (db_h, 1)]
            if EXACT:
                da_r = dprep.tile([128, NCH], F32, tag="dar32")
                db_r = dprep.tile([128, NCH], F32, tag="dbr32")
                nc.vector.tensor_tensor(da_r[:, :], da32[:, :], da_h[:, :], OP.subtract)
                nc.vector.tensor_tensor(db_r[:, :], db32[:, :], db_h[:, :], OP.subtract)
                da_rh = dprep.tile([128, NCH], BF16, tag="darh")
                db_rh = dprep.tile([128, NCH], BF16, tag="dbrh")
                nc.vector.tensor_copy(da_rh[:, :], da_r[:, :])
                nc.vector.tensor_copy(db_rh[:, :], db_r[:, :])
                parts += [(da_rh, 0), (db_rh, 1)]

            for e in range(E_PC):
                sr = estp.tile([128, NCH], I32, tag="sr")
                nc.sync.dma_start(sr[:, :], sr_d.ap()[e, :, :])
                hi = estp.tile([128, NCH], I16, tag="hi")
                lo = estp.tile([128, NCH], I16, tag="lo")
                hi32 = estp.tile([128, NCH], I32, tag="hi32")
                lo32 = estp.tile([128, NCH], I32, tag="lo32")
                nc.vector.tensor_single_scalar(hi32[:, :], sr[:, :], 9, OP.logical_shift_right)
                nc.vector.tensor_single_scalar(lo32[:, :], sr[:, :], 511, OP.bitwise_and)
                nc.vector.tensor_copy(hi[:, :], hi32[:, :])
                nc.vector.tensor_copy(lo[:, :], lo32[:, :])

                ps_a = psump.tile([128, 512], F32, tag="psa")
                ps_b = psump.tile([128, 512], F32, tag="psb")

                chunk = 0
                for g0 in range(0, NCH, G_BLK):
                    g = min(G_BLK, NCH - g0)
                    cmp = blkp.tile([128, G_BLK, 128], BF16, tag="cmp")
                    X = blkp.tile([128, G_BLK, 512], BF16, tag="X")
                    nc.vector.tensor_tensor(
                        cmp[:, :g, :],
                        hi[:, g0:g0 + g].unsqueeze(2).broadcast_to([128, g, 128]),
                        io128[:, :].unsqueeze(1).broadcast_to([128, g, 128]),
                        OP.is_equal)
                    nc.vector.tensor_tensor(
                        X[:, :g, :],
                        lo[:, g0:g0 + g].unsqueeze(2).broadcast_to([128, g, 512]),
                        io512[:, :].unsqueeze(1).broadcast_to([128, g, 512]),
                        OP.is_equal)
                    Ws = []
                    for pi, (val, tab) in enumerate(parts):
                        W = blkp.tile([128, G_BLK, 128], BF16, tag=f"W{pi}")
                        nc.vector.tensor_tensor(
                            W[:, :g, :],
                            cmp[:, :g, :],
                            val[:, g0:g0 + g].unsqueeze(2).broadcast_to([128, g, 128]),
                            OP.mult)
                        Ws.append((W, tab))
                    npart = {0: sum(1 for _, t in Ws if t == 0),
                             1: sum(1 for _, t in Ws if t == 1)}
                    for j in range(g):
                        first = chunk == 0
                        last = chunk == NCH - 1
                        seen = {0: 0, 1: 0}
                        for W, tab in Ws:
                            ps = ps_a if tab == 0 else ps_b
                            nc.tensor.matmul(
                                ps[:, :], W[:, j, :], X[:, j, :],
                                start=first and seen[tab] == 0,
                                stop=last and seen[tab] == npart[tab] - 1)
                            seen[tab] += 1
                        chunk += 1

                a_t = outp.tile([128, 512], F32, tag="a_in")
                b_t = outp.tile([128, 512], F32, tag="b_in")
                nc.sync.dma_start(a_t[:, :], a_d.ap()[e, :, :])
                nc.sync.dma_start(b_t[:, :], b_d.ap()[e, :, :])
                oa_t = outp.tile([128, 512], F32, tag="a_out")
                ob_t = outp.tile([128, 512], F32, tag="b_out")
                nc.vector.tensor_tensor(oa_t[:, :], a_t[:, :], ps_a[:, :], OP.add)
                nc.vector.tensor_tensor(ob_t[:, :], b_t[:, :], ps_b[:, :], OP.add)
                nc.sync.dma_start(oa_d.ap()[e, :, :], oa_t[:, :])
                nc.sync.dma_start(ob_d.ap()[e, :, :], ob_t[:, :])

    nc.compile()
    return nc


def _core_inputs(a, b, samples_regions, da, db, core):
    e0 = core * E_PC
    sr_c = samples_regions[:, e0:e0 + E_PC].astype(np.int32)
    sr_p = np.zeros((S_PAD, E_PC), np.int32)
    sr_p[:S] = sr_c
    da_p = np.zeros(S_PAD, np.float32); da_p[:S] = da
    db_p = np.zeros(S_PAD, np.float32); db_p[:S] = db
    return {
        "sr": sr_p.reshape(NCH, 128, E_PC).transpose(2, 1, 0).copy(),
        "da_l": da_p.reshape(NCH, 128).T.copy(),
        "db_l": db_p.reshape(NCH, 128).T.copy(),
        "a": np.ascontiguousarray(a[e0:e0 + E_PC]).reshape(E_PC, 128, 512).astype(np.float32),
        "b": np.ascontiguousarray(b[e0:e0 + E_PC]).reshape(E_PC, 128, 512).astype(np.float32),
        "iota128": np.tile(np.arange(128, dtype=np.int16), (128, 1)),
        "iota512": np.tile(np.arange(512, dtype=np.int16), (128, 1)),
    }


def kernel(a, b, samples_regions, da, db):
    global LAST_RESULTS, _CACHED_NC
    a = np.asarray(a); b = np.asarray(b)
    samples_regions = np.asarray(samples_regions)
    da = np.asarray(da); db = np.asarray(db)

    if _CACHED_NC is None:
        _CACHED_NC = _build_kernel()
    nc = _CACHED_NC

    in_maps = [_core_inputs(a, b, samples_regions, da, db, c)
               for c in range(N_CORES)]
    res = run_bass_kernel_spmd(nc, in_maps, core_ids=list(range(N_CORES)))
    LAST_RESULTS = res

    out = np.empty((2, E, R), np.float32)
    for c in range(N_CORES):
        e0 = c * E_PC
        out[0, e0:e0 + E_PC] = res.results[c]["out_a"].reshape(E_PC, R)
        out[1, e0:e0 + E_PC] = res.results[c]["out_b"].reshape(E_PC, R)
    return out


# revision 3
# speedup vs baseline: 1.7604x; 1.7604x over previous
"""Trainium2 Bass kernel for nn_EnsembleBeliefs (batched scatter-add into
per-estimator belief tables).

  new_a[e, r] = a[e, r] + sum_{s: samples_regions[s,e]==r} da[s]   (same for b)

Sharding: estimator-parallel across 8 NeuronCores (16 estimators each, no
cross-core communication).

Per-core algorithm (PE one-hot matmul scatter):
  region r = hi*512 + lo  (hi in [0,128) -> PSUM partition, lo in [0,512))
  For each 128-sample chunk (samples on SBUF partitions):
    W_da[s, h] = (hi_s == h) * da_s    fused tensor_scalar, fp16  [128, 128]
    X[s, l]    = (lo_s == l)           tensor_scalar one-hot, fp16 [128, 512]
    psum_a[h, l] += W_da^T @ X         TensorE matmul, fp32 PSUM accumulate
  After all chunks psum_a[h, l] holds sum of da over samples with
  idx == h*512 + l; out = a + psum_a.

fp16 carries da/db with a 11-bit significand (max rel err ~5e-4 on the
scattered increments); one-hots and products are exact. PSUM accumulation
is fp32. Set PARTS = 2 for full bf16 hi+lo splitting (~1e-6, 2x slower).
"""
import numpy as np
import concourse.bass as bass
import concourse.bacc as bacc
import concourse.tile as tile
from concourse import mybir
from concourse.bass_utils import run_bass_kernel_spmd

F32 = mybir.dt.float32
FP16 = mybir.dt.float16
BF16 = mybir.dt.bfloat16
I32 = mybir.dt.int32

E = 128          # estimators
R = 65536        # regions per estimator
S = 100000       # update samples
N_CORES = 8
E_PC = E // N_CORES          # 16 estimators per core
S_PAD = 100096               # S padded to a multiple of 128 (da/db padded with 0)
NCH = S_PAD // 128           # 782 sample chunks
PARTS = 1                    # 1: fp16 values; 2: bf16 hi+lo (exact, 2x matmuls)

LAST_RESULTS = None          # BassKernelResults of the most recent run
_CACHED_NC = None


def _build_kernel():
    nc = bacc.Bacc("TRN2", target_bir_lowering=False, debug=False,
                   num_devices=N_CORES)
    sr_d = nc.dram_tensor("sr", [E_PC, 128, NCH], I32, kind="ExternalInput")
    da_d = nc.dram_tensor("da_l", [128, NCH], F32, kind="ExternalInput")
    db_d = nc.dram_tensor("db_l", [128, NCH], F32, kind="ExternalInput")
    a_d = nc.dram_tensor("a", [E_PC, 128, 512], F32, kind="ExternalInput")
    b_d = nc.dram_tensor("b", [E_PC, 128, 512], F32, kind="ExternalInput")
    io128_d = nc.dram_tensor("iota128", [128, 128], FP16, kind="ExternalInput")
    io512_d = nc.dram_tensor("iota512", [128, 512], FP16, kind="ExternalInput")
    oa_d = nc.dram_tensor("out_a", [E_PC, 128, 512], F32, kind="ExternalOutput")
    ob_d = nc.dram_tensor("out_b", [E_PC, 128, 512], F32, kind="ExternalOutput")

    OP = mybir.AluOpType
    VDT = FP16 if PARTS == 1 else BF16

    with tile.TileContext(nc) as tc:
        with (
            tc.tile_pool(name="const", bufs=1) as constp,
            tc.tile_pool(name="dprep", bufs=1) as dprep,
            tc.tile_pool(name="est", bufs=2) as estp,
            tc.tile_pool(name="blk", bufs=4) as blkp,
            tc.tile_pool(name="outp", bufs=3) as outp,
            tc.tile_pool(name="psum", bufs=2, space=bass.MemorySpace.PSUM) as psump,
        ):
            io128 = constp.tile([128, 128], FP16)
            io512 = constp.tile([128, 512], FP16)
            nc.sync.dma_start(io128[:, :], io128_d.ap()[:, :])
            nc.sync.dma_start(io512[:, :], io512_d.ap()[:, :])

            # value streams: fp16 (PARTS=1) or bf16 hi+lo (PARTS=2)
            da32 = dprep.tile([128, NCH], F32, tag="d32")
            db32 = dprep.tile([128, NCH], F32, tag="d32b")
            nc.sync.dma_start(da32[:, :], da_d.ap()[:, :])
            nc.sync.dma_start(db32[:, :], db_d.ap()[:, :])
            parts = []   # (fp32 value tile for scalar2, table id) 0 = a, 1 = b
            if PARTS == 1:
                parts += [(da32, 0), (db32, 1)]
            else:
                # round to bf16 then upcast: hi part + residual, both exact
                da_h = dprep.tile([128, NCH], VDT, tag="dah")
                db_h = dprep.tile([128, NCH], VDT, tag="dbh")
                nc.vector.tensor_copy(da_h[:, :], da32[:, :])
                nc.vector.tensor_copy(db_h[:, :], db32[:, :])
                da_h32 = dprep.tile([128, NCH], F32, tag="dah32")
                db_h32 = dprep.tile([128, NCH], F32, tag="dbh32")
                nc.vector.tensor_copy(da_h32[:, :], da_h[:, :])
                nc.vector.tensor_copy(db_h32[:, :], db_h[:, :])
                da_r = dprep.tile([128, NCH], F32, tag="dar32")
                db_r = dprep.tile([128, NCH], F32, tag="dbr32")
                nc.vector.tensor_tensor(da_r[:, :], da32[:, :], da_h32[:, :], OP.subtract)
                nc.vector.tensor_tensor(db_r[:, :], db32[:, :], db_h32[:, :], OP.subtract)
                parts += [(da_h32, 0), (db_h32, 1), (da_r, 0), (db_r, 1)]

            n_per_tab = {0: sum(1 for _, t in parts if t == 0),
                         1: sum(1 for _, t in parts if t == 1)}

            for e in range(E_PC):
                sr = estp.tile([128, NCH], I32, tag="sr")
                nc.sync.dma_start(sr[:, :], sr_d.ap()[e, :, :])
                # hi/lo as fp16 (exact: values < 2048) so the one-hot
                # tensor_scalar ops run in the fast 16-bit perf mode.
                hi32 = estp.tile([128, NCH], I32, tag="hi32")
                lo32 = estp.tile([128, NCH], I32, tag="lo32")
                nc.vector.tensor_single_scalar(hi32[:, :], sr[:, :], 9, OP.logical_shift_right)
                nc.vector.tensor_single_scalar(lo32[:, :], sr[:, :], 511, OP.bitwise_and)
                hi = estp.tile([128, NCH], F32, tag="hi")
                lo = estp.tile([128, NCH], F32, tag="lo")
                nc.vector.tensor_copy(hi[:, :], hi32[:, :])
                nc.vector.tensor_copy(lo[:, :], lo32[:, :])

                ps_a = psump.tile([128, 512], F32, tag="psa")
                ps_b = psump.tile([128, 512], F32, tag="psb")

                for j in range(NCH):
                    first = j == 0
                    last = j == NCH - 1
                    X = blkp.tile([128, 512], FP16, tag="X")
                    nc.vector.tensor_scalar(
                        X[:, :], io512[:, :], lo[:, j:j + 1], None, OP.is_equal)
                    seen = {0: 0, 1: 0}
                    for pi, (val, tab) in enumerate(parts):
                        W = blkp.tile([128, 128], VDT, tag=f"W{pi}")
                        nc.vector.tensor_scalar(
                            W[:, :], io128[:, :], hi[:, j:j + 1], val[:, j:j + 1],
                            OP.is_equal, OP.mult)
                        ps = ps_a if tab == 0 else ps_b
                        nc.tensor.matmul(
                            ps[:, :], W[:, :], X[:, :],
                            start=first and seen[tab] == 0,
                            stop=last and seen[tab] == n_per_tab[tab] - 1)
                        seen[tab] += 1

                a_t = outp.tile([128, 512], F32, tag="a_in")
                b_t = outp.tile([128, 512], F32, tag="b_in")
                nc.sync.dma_start(a_t[:, :], a_d.ap()[e, :, :])
                nc.sync.dma_start(b_t[:, :], b_d.ap()[e, :, :])
                oa_t = outp.tile([128, 512], F32, tag="a_out")
                ob_t = outp.tile([128, 512], F32, tag="b_out")
                nc.vector.tensor_tensor(oa_t[:, :], a_t[:, :], ps_a[:, :], OP.add)
                nc.vector.tensor_tensor(ob_t[:, :], b_t[:, :], ps_b[:, :], OP.add)
                nc.sync.dma_start(oa_d.ap()[e, :, :], oa_t[:, :])
                nc.sync.dma_start(ob_d.ap()[e, :, :], ob_t[:, :])

    nc.compile()
    return nc


def _core_inputs(a, b, samples_regions, da, db, core):
    e0 = core * E_PC
    sr_c = samples_regions[:, e0:e0 + E_PC].astype(np.int32)
    sr_p = np.zeros((S_PAD, E_PC), np.int32)
    sr_p[:S] = sr_c
    da_p = np.zeros(S_PAD, np.float32); da_p[:S] = da
    db_p = np.zeros(S_PAD, np.float32); db_p[:S] = db
    return {
        "sr": sr_p.reshape(NCH, 128, E_PC).transpose(2, 1, 0).copy(),
        "da_l": da_p.reshape(NCH, 128).T.copy(),
        "db_l": db_p.reshape(NCH, 128).T.copy(),
        "a": np.ascontiguousarray(a[e0:e0 + E_PC]).reshape(E_PC, 128, 512).astype(np.float32),
        "b": np.ascontiguousarray(b[e0:e0 + E_PC]).reshape(E_PC, 128, 512).astype(np.float32),
        "iota128": np.tile(np.arange(128, dtype=np.float16), (128, 1)),
        "iota512": np.tile(np.arange(512, dtype=np.float16), (128, 1)),
    }


def kernel(a, b, samples_regions, da, db):
    global LAST_RESULTS, _CACHED_NC
    a = np.asarray(a); b = np.asarray(b)
    samples_regions = np.asarray(samples_regions)
    da = np.asarray(da); db = np.asarray(db)

    if _CACHED_NC is None:
        _CACHED_NC = _build_kernel()
    nc = _CACHED_NC

    in_maps = [_core_inputs(a, b, samples_regions, da, db, c)
               for c in range(N_CORES)]
    res = run_bass_kernel_spmd(nc, in_maps, core_ids=list(range(N_CORES)))
    LAST_RESULTS = res

    out = np.empty((2, E, R), np.float32)
    for c in range(N_CORES):
        e0 = c * E_PC
        out[0, e0:e0 + E_PC] = res.results[c]["out_a"].reshape(E_PC, R)
        out[1, e0:e0 + E_PC] = res.results[c]["out_b"].reshape(E_PC, R)
    return out


# revision 5
# speedup vs baseline: 2.0195x; 1.1471x over previous
"""Trainium2 Bass kernel for nn_EnsembleBeliefs (batched scatter-add into
per-estimator belief tables).

  new_a[e, r] = a[e, r] + sum_{s: samples_regions[s,e]==r} da[s]   (same for b)

Sharding: estimator-parallel across 8 NeuronCores (16 estimators each, no
cross-core communication).

Per-core algorithm (PE one-hot matmul scatter):
  region r = hi*512 + lo  (hi in [0,128) -> PSUM partition, lo in [0,512))
  For each 128-sample chunk (samples on SBUF partitions):
    W_da[s, h] = (hi_s == h) * da_s    fused tensor_scalar, fp16  [128, 128]
    X[s, l]    = (lo_s == l)           tensor_scalar one-hot, fp16 [128, 512]
    psum_a[h, l] += W_da^T @ X         TensorE matmul, fp32 PSUM accumulate
  After all chunks psum_a[h, l] holds sum of da over samples with
  idx == h*512 + l; out = a + psum_a.

fp16 carries da/db with a 11-bit significand (max rel err ~5e-4 on the
scattered increments); one-hots and products are exact. PSUM accumulation
is fp32. Set PARTS = 2 for full bf16 hi+lo splitting (~1e-6, 2x slower).
"""
import numpy as np
import concourse.bass as bass
import concourse.bacc as bacc
import concourse.tile as tile
from concourse import mybir
from concourse.bass_utils import run_bass_kernel_spmd

F32 = mybir.dt.float32
FP16 = mybir.dt.float16
BF16 = mybir.dt.bfloat16
I32 = mybir.dt.int32

E = 128          # estimators
R = 65536        # regions per estimator
S = 100000       # update samples
N_CORES = 8
E_PC = E // N_CORES          # 16 estimators per core
S_PAD = 100096               # S padded to a multiple of 128 (da/db padded with 0)
NCH = S_PAD // 128           # 782 sample chunks
G_BLK = 8                    # chunks per batched cmp/W build
PARTS = 1                    # 1: fp16 values; 2: bf16 hi+lo (exact, 2x matmuls)

LAST_RESULTS = None          # BassKernelResults of the most recent run
_CACHED_NC = None


def _build_kernel():
    nc = bacc.Bacc("TRN2", target_bir_lowering=False, debug=False,
                   num_devices=N_CORES)
    sr_d = nc.dram_tensor("sr", [E_PC, 128, NCH], I32, kind="ExternalInput")
    da_d = nc.dram_tensor("da_l", [128, NCH], F32, kind="ExternalInput")
    db_d = nc.dram_tensor("db_l", [128, NCH], F32, kind="ExternalInput")
    a_d = nc.dram_tensor("a", [E_PC, 128, 512], F32, kind="ExternalInput")
    b_d = nc.dram_tensor("b", [E_PC, 128, 512], F32, kind="ExternalInput")
    io128r_d = nc.dram_tensor("iota128r", [128, 128 * G_BLK], FP16, kind="ExternalInput")
    io512_d = nc.dram_tensor("iota512", [128, 512], FP16, kind="ExternalInput")
    oa_d = nc.dram_tensor("out_a", [E_PC, 128, 512], F32, kind="ExternalOutput")
    ob_d = nc.dram_tensor("out_b", [E_PC, 128, 512], F32, kind="ExternalOutput")

    OP = mybir.AluOpType
    VDT = FP16 if PARTS == 1 else BF16

    with tile.TileContext(nc) as tc:
        with (
            tc.tile_pool(name="const", bufs=1) as constp,
            tc.tile_pool(name="dprep", bufs=1) as dprep,
            tc.tile_pool(name="est", bufs=2) as estp,
            tc.tile_pool(name="blk", bufs=4) as blkp,
            tc.tile_pool(name="outp", bufs=3) as outp,
            tc.tile_pool(name="psum", bufs=2, space=bass.MemorySpace.PSUM) as psump,
        ):
            io128r = constp.tile([128, 128, G_BLK], FP16)   # io128r[p, h, j] = h
            io512 = constp.tile([128, 512], FP16)
            nc.sync.dma_start(io128r[:, :, :], io128r_d.ap()[:, :].rearrange("p (h j) -> p h j", j=G_BLK))
            nc.sync.dma_start(io512[:, :], io512_d.ap()[:, :])

            # value streams: fp16 (PARTS=1) or bf16 hi+lo (PARTS=2)
            da32 = dprep.tile([128, NCH], F32, tag="d32")
            db32 = dprep.tile([128, NCH], F32, tag="d32b")
            nc.sync.dma_start(da32[:, :], da_d.ap()[:, :])
            nc.sync.dma_start(db32[:, :], db_d.ap()[:, :])
            parts = []   # (fp16/bf16 value tile, table id) 0 = a, 1 = b
            if PARTS == 1:
                da16 = dprep.tile([128, NCH], FP16, tag="da16")
                db16 = dprep.tile([128, NCH], FP16, tag="db16")
                nc.vector.tensor_copy(da16[:, :], da32[:, :])
                nc.vector.tensor_copy(db16[:, :], db32[:, :])
                parts += [(da16, 0), (db16, 1)]
            else:
                # round to bf16 then upcast: hi part + residual, both exact
                da_h = dprep.tile([128, NCH], VDT, tag="dah")
                db_h = dprep.tile([128, NCH], VDT, tag="dbh")
                nc.vector.tensor_copy(da_h[:, :], da32[:, :])
                nc.vector.tensor_copy(db_h[:, :], db32[:, :])
                da_h32 = dprep.tile([128, NCH], F32, tag="dah32")
                db_h32 = dprep.tile([128, NCH], F32, tag="dbh32")
                nc.vector.tensor_copy(da_h32[:, :], da_h[:, :])
                nc.vector.tensor_copy(db_h32[:, :], db_h[:, :])
                da_r = dprep.tile([128, NCH], F32, tag="dar32")
                db_r = dprep.tile([128, NCH], F32, tag="dbr32")
                nc.vector.tensor_tensor(da_r[:, :], da32[:, :], da_h32[:, :], OP.subtract)
                nc.vector.tensor_tensor(db_r[:, :], db32[:, :], db_h32[:, :], OP.subtract)
                da_rh = dprep.tile([128, NCH], VDT, tag="darh")
                db_rh = dprep.tile([128, NCH], VDT, tag="dbrh")
                nc.vector.tensor_copy(da_rh[:, :], da_r[:, :])
                nc.vector.tensor_copy(db_rh[:, :], db_r[:, :])
                parts += [(da_h, 0), (db_h, 1), (da_rh, 0), (db_rh, 1)]

            n_per_tab = {0: sum(1 for _, t in parts if t == 0),
                         1: sum(1 for _, t in parts if t == 1)}

            for e in range(E_PC):
                sr = estp.tile([128, NCH], I32, tag="sr")
                nc.sync.dma_start(sr[:, :], sr_d.ap()[e, :, :])
                # hi/lo as fp16 (exact: values < 2048) so the one-hot
                # tensor_scalar ops run in the fast 16-bit perf mode.
                hi32 = estp.tile([128, NCH], I32, tag="hi32")
                lo32 = estp.tile([128, NCH], I32, tag="lo32")
                nc.vector.tensor_single_scalar(hi32[:, :], sr[:, :], 9, OP.logical_shift_right)
                nc.vector.tensor_single_scalar(lo32[:, :], sr[:, :], 511, OP.bitwise_and)
                hi16 = estp.tile([128, NCH], FP16, tag="hi16")
                lo = estp.tile([128, NCH], F32, tag="lo")
                nc.vector.tensor_copy(hi16[:, :], hi32[:, :])
                nc.vector.tensor_copy(lo[:, :], lo32[:, :])

                ps_a = psump.tile([128, 512], F32, tag="psa")
                ps_b = psump.tile([128, 512], F32, tag="psb")

                for g0 in range(0, NCH, G_BLK):
                    g = min(G_BLK, NCH - g0)
                    # cmp[p, h, j] = (hi[p, g0+j] == h), inner dim j step-1
                    cmp = blkp.tile([128, 128, G_BLK], FP16, tag="cmp")
                    nc.vector.tensor_tensor(
                        cmp[:, :, :g],
                        hi16[:, g0:g0 + g].unsqueeze(1).broadcast_to([128, 128, g]),
                        io128r[:, :, :g],
                        OP.is_equal)
                    Ws = []
                    for pi, (val, tab) in enumerate(parts):
                        W = blkp.tile([128, 128, G_BLK], VDT, tag=f"W{pi}")
                        eng = nc.gpsimd if pi % 2 == 1 else nc.vector
                        eng.tensor_tensor(
                            W[:, :, :g],
                            cmp[:, :, :g],
                            val[:, g0:g0 + g].unsqueeze(1).broadcast_to([128, 128, g]),
                            OP.mult)
                        Ws.append((W, tab))
                    for j in range(g):
                        ch = g0 + j
                        first = ch == 0
                        last = ch == NCH - 1
                        X = blkp.tile([128, 512], FP16, tag="X")
                        nc.vector.tensor_scalar(
                            X[:, :], io512[:, :], lo[:, ch:ch + 1], None, OP.is_equal)
                        seen = {0: 0, 1: 0}
                        for W, tab in Ws:
                            ps = ps_a if tab == 0 else ps_b
                            nc.tensor.matmul(
                                ps[:, :], W[:, :, j], X[:, :],
                                start=first and seen[tab] == 0,
                                stop=last and seen[tab] == n_per_tab[tab] - 1)
                            seen[tab] += 1

                a_t = outp.tile([128, 512], F32, tag="a_in")
                b_t = outp.tile([128, 512], F32, tag="b_in")
                nc.sync.dma_start(a_t[:, :], a_d.ap()[e, :, :])
                nc.sync.dma_start(b_t[:, :], b_d.ap()[e, :, :])
                oa_t = outp.tile([128, 512], F32, tag="a_out")
                ob_t = outp.tile([128, 512], F32, tag="b_out")
                nc.vector.tensor_tensor(oa_t[:, :], a_t[:, :], ps_a[:, :], OP.add)
                nc.vector.tensor_tensor(ob_t[:, :], b_t[:, :], ps_b[:, :], OP.add)
                nc.sync.dma_start(oa_d.ap()[e, :, :], oa_t[:, :])
                nc.sync.dma_start(ob_d.ap()[e, :, :], ob_t[:, :])

    nc.compile()
    return nc


def _core_inputs(a, b, samples_regions, da, db, core):
    e0 = core * E_PC
    sr_c = samples_regions[:, e0:e0 + E_PC].astype(np.int32)
    sr_p = np.zeros((S_PAD, E_PC), np.int32)
    sr_p[:S] = sr_c
    da_p = np.zeros(S_PAD, np.float32); da_p[:S] = da
    db_p = np.zeros(S_PAD, np.float32); db_p[:S] = db
    return {
        "sr": sr_p.reshape(NCH, 128, E_PC).transpose(2, 1, 0).copy(),
        "da_l": da_p.reshape(NCH, 128).T.copy(),
        "db_l": db_p.reshape(NCH, 128).T.copy(),
        "a": np.ascontiguousarray(a[e0:e0 + E_PC]).reshape(E_PC, 128, 512).astype(np.float32),
        "b": np.ascontiguousarray(b[e0:e0 + E_PC]).reshape(E_PC, 128, 512).astype(np.float32),
        "iota128r": np.tile(np.repeat(np.arange(128, dtype=np.float16), G_BLK), (128, 1)),
        "iota512": np.tile(np.arange(512, dtype=np.float16), (128, 1)),
    }


def kernel(a, b, samples_regions, da, db):
    global LAST_RESULTS, _CACHED_NC
    a = np.asarray(a); b = np.asarray(b)
    samples_regions = np.asarray(samples_regions)
    da = np.asarray(da); db = np.asarray(db)

    if _CACHED_NC is None:
        _CACHED_NC = _build_kernel()
    nc = _CACHED_NC

    in_maps = [_core_inputs(a, b, samples_regions, da, db, c)
               for c in range(N_CORES)]
    res = run_bass_kernel_spmd(nc, in_maps, core_ids=list(range(N_CORES)))
    LAST_RESULTS = res

    out = np.empty((2, E, R), np.float32)
    for c in range(N_CORES):
        e0 = c * E_PC
        out[0, e0:e0 + E_PC] = res.results[c]["out_a"].reshape(E_PC, R)
        out[1, e0:e0 + E_PC] = res.results[c]["out_b"].reshape(E_PC, R)
    return out
